# revision 32
# baseline (speedup 1.0000x reference)
"""LocalWindowAttention Trainium2 kernel.

Strategy: data-parallel over the 1024 (B*n_windows) windows -> 8 cores x 128
windows (2048 tokens each). All four projections run as fp8e4m3 DoubleRow
matmuls (0.5 cycles/row, K=256 per instruction) with an error-compensated
hi/lo split of both operands:

    w @ x ~= w_hi@x_hi + (w_hi@x_lo + w_lo@x_hi)

The hi*hi pass consumes k-tile PAIRS per DoubleRow instruction; the two cross
terms pack into the two DoubleRow slots of a single instruction (slot0 =
w_hi*x_lo, slot1 = w_lo*x_hi). Total K-volume = 3x the fp32 contraction at 4x
rate -> 0.75x bf16 cost, rel-err ~1.4e-3 per projection.

Weights are pre-scaled by A=32 on the host so their hi/lo parts stay in
e4m3's normal range; descale folds into the RoPE tables (Q/K), rides
through V (vt = A*v) and is removed at the final 1/A^2 output copy.

Software pipelining: the attention of block b (DVE/ACT-heavy softmax chain,
PE-light) is interleaved into the Q/K/V projection units of block b+1 so the
tensor engine never drains. Each attention quad is split into part1 (scores
+ softmax issue) and part2 (transpose + AV + fp8 re-quantize) issued ~2
projection units later so the vector-chain latency hides under matmuls.
"""

import json
import os
from functools import lru_cache

import numpy as np
import ml_dtypes

import concourse.bass as bass
import concourse.mybir as mybir
import concourse.tile as tile
from concourse.bass_utils import run_bass_kernel_spmd


def _split_waits_json(bir: bytes) -> bytes:
    """Walrus in this container embeds at most 1 sem-wait per instruction
    (2 for EventSemaphore). Tile freely attaches more. Spill the excess
    onto same-engine NoOps inserted right before the instruction."""
    j = json.loads(bir)
    ctr = [0]

    def cap_of(op):
        return 2 if op == "EventSemaphore" else 1

    for f in j["functions"]:
        for blk in f["blocks"]:
            out = []
            for inst in blk["instructions"]:
                si = inst.get("sync_info")
                waits = (si or {}).get("on_wait") or []
                cap = cap_of(inst.get("opcode"))
                if len(waits) > cap:
                    extra, keep = waits[:-cap], waits[-cap:]
                    for w in extra:
                        ctr[0] += 1
                        out.append({
                            "debug": inst.get("debug", 0),
                            "engine": inst["engine"],
                            "ins": [], "outs": [],
                            "name": f"I-wspill-{ctr[0]}",
                            "opcode": "NoOp",
                            "sync_info": {"on_update": [], "on_wait": [w]},
                        })
                    si["on_wait"] = keep
                out.append(inst)
            blk["instructions"] = out
    return json.dumps(j).encode()


def _patch_to_json(nc):
    orig = nc.to_json_bytes
    nc.to_json_bytes = lambda: _split_waits_json(orig())
    return nc

F32 = mybir.dt.float32
F8 = mybir.dt.float8e4
BF16 = mybir.dt.bfloat16
AX = mybir.AxisListType
ALU = mybir.AluOpType
ACTF = mybir.ActivationFunctionType
DR = mybir.MatmulPerfMode.DoubleRow

B, S, D = 4, 4096, 2048
H, HD, W = 16, 128, 16
E = H * HD  # 2048
NCORES = 8
TOK_PER_CORE = B * S // NCORES  # 2048
TBLK = 512            # tokens per block
NBLK = TOK_PER_CORE // TBLK  # 4
KT = D // 128         # 16 contraction tiles
ET = E // 128         # 16 e-tiles (= heads)
NG = TBLK // 128      # 4 groups (of 8 windows) per block
MASK_NEG = -30000.0
A_SCALE = 32.0        # weight pre-scale so w_lo stays in e4m3 normal range


def build_kernel(nblk=NBLK):
    nc = bass.Bass("TRN2", target_bir_lowering=False, debug=False)

    ntok = nblk * TBLK
    # DRAM I/O (per core). fp8 operands come as interleaved hi/lo pairs:
    # x/o slots = (lo, hi); weight slots = (hi, lo) -> a "cross" DoubleRow
    # instruction pairing both slot-dims yields w_hi@x_lo + w_lo@x_hi.
    xcd = nc.dram_tensor("xc", [nblk, 128, KT, 2, TBLK], F8, kind="ExternalInput")
    wqcd = nc.dram_tensor("wqc", [ET, 128, KT, 2, 128], F8, kind="ExternalInput")
    wkcd = nc.dram_tensor("wkc", [ET, 128, KT, 2, 128], F8, kind="ExternalInput")
    wvcd = nc.dram_tensor("wvc", [E // 512, 128, KT, 2, 512], F8, kind="ExternalInput")
    wocd = nc.dram_tensor("woc", [D // 512, 128, ET, 2, 512], F8, kind="ExternalInput")
    csq = nc.dram_tensor("csq", [128, TBLK], BF16, kind="ExternalInput")
    snq = nc.dram_tensor("snq", [128, TBLK], BF16, kind="ExternalInput")
    csk = nc.dram_tensor("csk", [128, TBLK], BF16, kind="ExternalInput")
    snk = nc.dram_tensor("snk", [128, TBLK], BF16, kind="ExternalInput")
    maskd = nc.dram_tensor("maskd", [128, 4, 128], F32, kind="ExternalInput")
    idend = nc.dram_tensor("idend", [128, 128], BF16, kind="ExternalInput")
    outd = nc.dram_tensor("out", [ntok, D], BF16, kind="ExternalOutput")

    with tile.TileContext(nc) as tc:
        with (
            tc.tile_pool(name="const", bufs=1) as constp,
            tc.tile_pool(name="x", bufs=1) as xpool,
            tc.tile_pool(name="wqk", bufs=4) as wqkp,
            tc.tile_pool(name="wvo", bufs=3) as wvop,
            tc.tile_pool(name="qk", bufs=2) as qkpool,
            tc.tile_pool(name="v", bufs=1) as vpool,
            tc.tile_pool(name="oc", bufs=1) as ocpool,
            tc.tile_pool(name="rope", bufs=2) as ropep,
            tc.tile_pool(name="attn", bufs=2) as attnp,
            tc.tile_pool(name="small", bufs=4) as smallp,
            tc.tile_pool(name="osb", bufs=2) as osbp,
            tc.tile_pool(name="psA", bufs=3, space="PSUM") as psA,
            tc.tile_pool(name="psS", bufs=2, space="PSUM") as psS,
            tc.tile_pool(name="psT", bufs=1, space="PSUM") as psT,
            tc.tile_pool(name="psO", bufs=2, space="PSUM") as psO,
        ):
            # constants
            cs_q = constp.tile([128, TBLK], BF16, tag="csq")
            sn_q = constp.tile([128, TBLK], BF16, tag="snq")
            cs_k = constp.tile([128, TBLK], BF16, tag="csk")
            sn_k = constp.tile([128, TBLK], BF16, tag="snk")
            mask = constp.tile([128, 4, 128], F32, tag="mask")
            iden = constp.tile([128, 128], BF16, tag="iden")

            def make_qk(xt, qrt, krt):
                """Return (preload, units): the first PRE weight loads can
                be issued early (previous iteration) via preload(); each
                unit issues the load for unit i+PRE, 24 DoubleRow matmuls,
                and the RoPE copy-out. 32 units per block."""
                PRE = 3
                seq = []
                for wdram, cs_t, sn_t, dest in (
                    (wqcd, cs_q, sn_q, qrt),
                    (wkcd, cs_k, sn_k, krt),
                ):
                    for et in range(ET):
                        seq.append((wdram, cs_t, sn_t, dest, et))
                tiles = {}

                def load(i):
                    wt = wqkp.tile([128, KT, 2, 128], F8, tag="wqk")
                    nc.scalar.dma_start(wt[:], seq[i][0][seq[i][4]])
                    tiles[i] = wt

                def preload():
                    for i in range(PRE):
                        load(i)

                def units():
                    for i, (wdram, cs_t, sn_t, dest, et) in enumerate(seq):
                        def unit(i=i, cs_t=cs_t, sn_t=sn_t, dest=dest, et=et):
                            if i + PRE < len(seq):
                                load(i + PRE)
                            wt = tiles.pop(i)
                            ps = psA.tile([128, TBLK], F32, tag="proj")
                            for j in range(KT // 2):
                                nc.tensor.matmul(
                                    ps[:], wt[:, 2 * j:2 * j + 2, 0, :],
                                    xt[:, 2 * j:2 * j + 2, 1, :],
                                    start=(j == 0), stop=False, perf_mode=DR)
                            for k in range(KT):
                                nc.tensor.matmul(
                                    ps[:], wt[:, k, :, :], xt[:, k, :, :],
                                    start=False, stop=(k == KT - 1),
                                    perf_mode=DR)
                            # RoPE in bf16: qb = bf16(ps) on ACT, then
                            # dest = qb*cs + swap64(qb)*sn on DVE (2x mode)
                            qb = ropep.tile([128, TBLK], BF16, tag="qb")
                            nc.scalar.activation(qb[:], ps[:], ACTF.Copy)
                            # sn tables are half-swapped on host so in0/in1
                            # share a base partition (walrus SB constraint)
                            rot = ropep.tile([128, TBLK], BF16, tag="rot")
                            qcs = ropep.tile([128, TBLK], BF16, tag="qcs")
                            nc.vector.tensor_tensor(
                                out=rot[0:64, :], in0=qb[64:128, :],
                                in1=sn_t[64:128, :], op=ALU.mult)
                            nc.vector.tensor_tensor(
                                out=rot[64:128, :], in0=qb[0:64, :],
                                in1=sn_t[0:64, :], op=ALU.mult)
                            nc.vector.tensor_tensor(
                                out=qcs[:], in0=qb[:], in1=cs_t[:],
                                op=ALU.mult)
                            nc.vector.tensor_tensor(
                                out=dest[:, et, :], in0=qcs[:], in1=rot[:],
                                op=ALU.add)
                        yield unit

                return preload, units

            def v_units(xt, vt):
                """Yield one closure per (ec, tt) V-projection psum tile.
                Weight chunk ec+1 is prefetched at (ec, tt=0); loads are
                split in halves so matmuls start on the first piece."""
                tiles = {}

                def load(ec):
                    wv = wvop.tile([128, KT, 2, 512], F8, tag="wvo")
                    for q in range(4):
                        nc.scalar.dma_start(
                            wv[:, q * 4:(q + 1) * 4, :, :],
                            wvcd[ec][:, q * 4:(q + 1) * 4, :, :])
                    tiles[ec] = wv

                for ec in range(E // 512):
                    for tt in range(NG):
                        def unit(ec=ec, tt=tt):
                            if ec == 0 and tt == 0:
                                load(0)
                                load(1)
                            if tt == 0 and ec + 2 < E // 512:
                                load(ec + 2)
                            wv = tiles[ec]
                            ps = psA.tile([128, 512], F32, tag="proj")
                            for j in range(KT // 2):
                                nc.tensor.matmul(
                                    ps[:],
                                    xt[:, 2 * j:2 * j + 2, 1,
                                       tt * 128:(tt + 1) * 128],
                                    wv[:, 2 * j:2 * j + 2, 0, :],
                                    start=(j == 0), stop=False, perf_mode=DR)
                            for k in range(KT):
                                nc.tensor.matmul(
                                    ps[:],
                                    xt[:, k, :, tt * 128:(tt + 1) * 128],
                                    wv[:, k, :, :],
                                    start=False, stop=(k == KT - 1),
                                    perf_mode=DR)
                            nc.scalar.activation(
                                vt[:, tt, ec * 512:(ec + 1) * 512], ps[:],
                                ACTF.Copy)
                        yield unit

            def attn_quads(qrt, krt, vt, oct_):
                """Yield (part1, part2) closures per (g, h0) quad."""
                for g in range(NG):
                    gs = g * 128
                    for h0 in range(0, H, 4):
                        state = {}

                        def part1(g=g, gs=gs, h0=h0, state=state):
                            sps = psS.tile([128, 4, 128], F32, tag="s")
                            for i in range(4):
                                h = h0 + i
                                nc.tensor.matmul(
                                    sps[:, i, :], qrt[:, h, gs:gs + 128],
                                    krt[:, h, gs:gs + 128],
                                    start=True, stop=True)
                            sm = attnp.tile([128, 4, 128], F32, tag="sm")
                            nc.vector.tensor_tensor(
                                out=sm[:], in0=sps[:], in1=mask[:],
                                op=ALU.add)
                            pt = attnp.tile([128, 4, 128], BF16, tag="pt")
                            sums = smallp.tile([128, 4], F32, tag="sums")
                            for i in range(4):
                                nc.scalar.activation(
                                    pt[:, i, :], sm[:, i, :], ACTF.Exp,
                                    accum_out=sums[:, i:i + 1])
                            rec = smallp.tile([128, 4], F32, tag="rec")
                            nc.vector.reciprocal(rec[:], sums[:])
                            at = attnp.tile([128, 4, 128], BF16, tag="at")
                            for i in range(4):
                                nc.vector.tensor_scalar_mul(
                                    at[:, i, :], pt[:, i, :], rec[:, i:i + 1])
                            state["at"] = at

                        def part2(g=g, gs=gs, h0=h0, state=state):
                            at = state["at"]
                            atps = psT.tile([128, 4, 128], BF16, tag="t")
                            for i in range(4):
                                nc.tensor.transpose(
                                    atps[:, i, :], at[:, i, :], iden[:])
                            ats = attnp.tile([128, 4, 128], BF16, tag="ats")
                            nc.vector.tensor_copy(ats[:], atps[:])
                            ops_ = psO.tile([128, 4, 128], F32, tag="o")
                            for i in range(4):
                                h = h0 + i
                                nc.tensor.matmul(
                                    ops_[:, i, :],
                                    vt[:, g, h * 128:(h + 1) * 128],
                                    ats[:, i, :], start=True, stop=True)
                            # quantize A*outT to fp8 pair: hi then lo residual
                            nc.scalar.activation(
                                oct_[:, h0:h0 + 4, 1, gs:gs + 128], ops_[:],
                                ACTF.Copy)
                            nc.vector.tensor_tensor(
                                out=oct_[:, h0:h0 + 4, 0, gs:gs + 128],
                                in0=ops_[:],
                                in1=oct_[:, h0:h0 + 4, 1, gs:gs + 128],
                                op=ALU.subtract)

                        yield part1, part2

            def o_units(oct_, ts):
                """Yield one closure per (dc, tt) O-projection psum tile."""
                tiles = {}

                def load(dc):
                    wo = wvop.tile([128, ET, 2, 512], F8, tag="wvo")
                    for q in range(4):
                        nc.scalar.dma_start(
                            wo[:, q * 4:(q + 1) * 4, :, :],
                            wocd[dc][:, q * 4:(q + 1) * 4, :, :])
                    tiles[dc] = wo

                for dc in range(D // 512):
                    for tt in range(NG):
                        def unit(dc=dc, tt=tt):
                            if dc == 0 and tt == 0:
                                load(0)
                                load(1)
                            if tt == 0 and dc + 2 < D // 512:
                                load(dc + 2)
                            wo = tiles[dc]
                            ps = psA.tile([128, 512], F32, tag="proj")
                            for j in range(ET // 2):
                                nc.tensor.matmul(
                                    ps[:],
                                    oct_[:, 2 * j:2 * j + 2, 1,
                                         tt * 128:(tt + 1) * 128],
                                    wo[:, 2 * j:2 * j + 2, 0, :],
                                    start=(j == 0), stop=False, perf_mode=DR)
                            for et in range(ET):
                                nc.tensor.matmul(
                                    ps[:],
                                    oct_[:, et, :, tt * 128:(tt + 1) * 128],
                                    wo[:, et, :, :],
                                    start=False, stop=(et == ET - 1),
                                    perf_mode=DR)
                            osb = osbp.tile([128, 512], BF16, tag="osb")
                            nc.scalar.activation(
                                osb[:], ps[:], ACTF.Copy,
                                scale=1.0 / (A_SCALE * A_SCALE))
                            nc.sync.dma_start(
                                outd[ts + tt * 128: ts + (tt + 1) * 128,
                                     dc * 512:(dc + 1) * 512],
                                osb[:],
                            )
                        yield unit

            def load_x(b):
                xt = xpool.tile([128, KT, 2, TBLK], F8, tag="xc")
                for q in range(4):
                    nc.sync.dma_start(
                        xt[:, q * 4:(q + 1) * 4, :, :],
                        xcd[b][:, q * 4:(q + 1) * 4, :, :])
                return xt

            # x(0) ahead of the constants so the first matmuls start early
            xt_next = load_x(0)
            nc.sync.dma_start(cs_q[:], csq[:])
            nc.sync.dma_start(sn_q[:], snq[:])
            nc.sync.dma_start(cs_k[:], csk[:])
            nc.sync.dma_start(sn_k[:], snk[:])
            nc.sync.dma_start(mask[:], maskd[:])
            nc.sync.dma_start(iden[:], idend[:])

            def prep_block(bb):
                """Create block bb's q/k tiles + weight preloader."""
                qrt = qkpool.tile([128, ET, TBLK], BF16, tag="qrt")
                krt = qkpool.tile([128, ET, TBLK], BF16, tag="krt")
                pre, us = make_qk(xt_next, qrt, krt)
                pre()
                return qrt, krt, us

            # ---- software-pipelined block loop: attention of block b-1
            # rides under the projection units of block b; the last block's
            # attention rides under its own O-projection.
            nxt = prep_block(0)
            prev = None  # ((qrt, krt, vt, oct_), oct_, ts_prev)
            for b in range(nblk + 1):
                units = []
                if b < nblk:
                    xt = xt_next
                    qrt, krt, qk_us = nxt
                    vt = vpool.tile([128, NG, E], BF16, tag="vt")
                    units.extend(qk_us())
                    units.extend(v_units(xt, vt))

                if prev is not None:
                    quads = list(attn_quads(*prev[0]))
                    o_us = list(o_units(prev[1], prev[2]))
                    if units:
                        pend = []
                        qi = 0
                        for ui, unit in enumerate(units):
                            unit()
                            if ui % 3 == 2:
                                if len(pend) >= 2:
                                    pend.pop(0)[1]()
                                if qi < len(quads):
                                    quads[qi][0]()
                                    pend.append(quads[qi])
                                    qi += 1
                        while qi < len(quads) or pend:
                            if pend:
                                pend.pop(0)[1]()
                            if qi < len(quads):
                                quads[qi][0]()
                                pend.append(quads[qi])
                                qi += 1
                        if b + 1 < nblk:
                            xt_next = load_x(b + 1)
                            nxt = prep_block(b + 1)
                        for u in o_us:
                            u()
                    else:
                        # final iteration: interleave the quads with the
                        # first wo chunk's O units (unit dc0/tt=g becomes
                        # ready once group g's quads complete).
                        pend = []
                        done = 0
                        for qi, quad in enumerate(quads):
                            quad[0]()
                            if pend:
                                pend.pop(0)[1]()
                                done += 1
                                if done % 4 == 0:
                                    o_us[done // 4 - 1]()
                            pend.append(quad)
                        while pend:
                            pend.pop(0)[1]()
                            done += 1
                            if done % 4 == 0:
                                o_us[done // 4 - 1]()
                        for u in o_us[NG:]:
                            u()
                else:
                    for unit in units:
                        unit()
                    if nblk > 1:
                        xt_next = load_x(1)
                        nxt = prep_block(1)

                if b < nblk:
                    oct_ = ocpool.tile([128, ET, 2, TBLK], F8, tag="oc")
                    prev = ((qrt, krt, vt, oct_), oct_, b * TBLK)
    return _patch_to_json(nc)


_F8NP = ml_dtypes.float8_e4m3


def _split_f8(a):
    hi = a.astype(_F8NP)
    lo = (a - hi.astype(np.float32)).astype(_F8NP)
    return hi, lo


def _interleave(arr, lohi):
    """arr [KTlike, 128, C0, C1] fp32 -> [C0, 128, KTlike, 2, C1] fp8 with
    slot order (hi, lo) when lohi=False else (lo, hi)."""
    hi, lo = _split_f8(arr)
    pair = (lo, hi) if lohi else (hi, lo)
    st = np.stack(pair, axis=-1)  # [KT, 128, C0, C1, 2]
    return np.ascontiguousarray(st.transpose(2, 1, 0, 4, 3))


def _host_prep(x, rope_freqs, wq, wk, wv, wo):
    A = A_SCALE
    x_flat = np.ascontiguousarray(x, dtype=np.float32).reshape(B * S, D)

    def w_c(w, c1):
        wT = np.ascontiguousarray(w.T, dtype=np.float32) * A
        kt = wT.shape[0] // 128
        arr = wT.reshape(kt, 128, wT.shape[1] // c1, c1)
        return _interleave(arr, lohi=False)

    wqc = w_c(wq, 128)
    wkc = w_c(wk, 128)
    wvc = w_c(wv, 512)
    woc = w_c(wo, 512)

    f = np.asarray(rope_freqs[:W], dtype=np.float32)  # [16, 64]
    cosf, sinf = np.cos(f), np.sin(f)                 # [16, 64]
    tmod = np.arange(TBLK) % W
    cs = np.empty((128, TBLK), np.float32)
    sn = np.empty((128, TBLK), np.float32)
    p = np.arange(128)
    cs[:, :] = cosf[tmod[None, :], (p % 64)[:, None]]
    sn[:, :] = sinf[tmod[None, :], (p % 64)[:, None]]
    sn[0:64, :] *= -1.0
    scale = 1.0 / np.sqrt(np.float32(HD))
    bf = ml_dtypes.bfloat16
    # halves pre-swapped: kernel computes rot[0:64] = qb[64:128]*sn_sw[64:128]
    sn_sw = np.concatenate([sn[64:128], sn[0:64]], axis=0)
    csq, snq = (cs * (scale / A)).astype(bf), (sn_sw * (scale / A)).astype(bf)
    csk, snk = (cs * (1.0 / A)).astype(bf), (sn_sw * (1.0 / A)).astype(bf)

    maskm = np.full((128, 128), MASK_NEG, np.float32)
    for wdw in range(128 // W):
        maskm[wdw * W:(wdw + 1) * W, wdw * W:(wdw + 1) * W] = 0.0
    maskm = np.repeat(maskm[:, None, :], 4, axis=1).copy()
    iden = np.eye(128, dtype=ml_dtypes.bfloat16)

    shared = dict(wqc=wqc, wkc=wkc, wvc=wvc, woc=woc,
                  csq=csq, snq=snq, csk=csk, snk=snk,
                  maskd=maskm, idend=iden)
    in_maps = []
    for c in range(NCORES):
        shard = x_flat[c * TOK_PER_CORE:(c + 1) * TOK_PER_CORE]
        xT = np.ascontiguousarray(shard.T)  # [D, 2048]
        arr = xT.reshape(KT, 128, NBLK, TBLK)
        xc = _interleave(arr, lohi=True)  # [NBLK, 128, KT, 2, TBLK]
        in_maps.append(dict(shared, xc=xc))
    return in_maps


@lru_cache(maxsize=1)
def _get_nc():
    return build_kernel()


def kernel(x, rope_freqs, wq, wk, wv, wo):
    in_maps = _host_prep(x, rope_freqs, wq, wk, wv, wo)
    nc = _get_nc()
    res = run_bass_kernel_spmd(
        nc, in_maps, core_ids=list(range(NCORES)),
        trace=bool(int(os.environ.get("LWA_TRACE", "0"))),
    )
    if getattr(kernel, "_last_results", None) is not None or True:
        kernel._last_results = res
    out = np.concatenate(
        [np.asarray(r["out"], dtype=np.float32) for r in res.results], axis=0)
    return out.reshape(B, S, D)


# revision 33
# speedup vs baseline: 1.0039x; 1.0039x over previous
"""LocalWindowAttention Trainium2 kernel.

Strategy: data-parallel over the 1024 (B*n_windows) windows -> 8 cores x 128
windows (2048 tokens each). All four projections run as fp8e4m3 DoubleRow
matmuls (0.5 cycles/row, K=256 per instruction) with an error-compensated
hi/lo split of both operands:

    w @ x ~= w_hi@x_hi + (w_hi@x_lo + w_lo@x_hi)

The hi*hi pass consumes k-tile PAIRS per DoubleRow instruction; the two cross
terms pack into the two DoubleRow slots of a single instruction (slot0 =
w_hi*x_lo, slot1 = w_lo*x_hi). Total K-volume = 3x the fp32 contraction at 4x
rate -> 0.75x bf16 cost, rel-err ~1.4e-3 per projection.

Weights are pre-scaled by A=32 on the host so their hi/lo parts stay in
e4m3's normal range; descale folds into the RoPE tables (Q/K), rides
through V (vt = A*v) and is removed at the final 1/A^2 output copy.

Software pipelining: the attention of block b (DVE/ACT-heavy softmax chain,
PE-light) is interleaved into the Q/K/V projection units of block b+1 so the
tensor engine never drains. Each attention quad is split into part1 (scores
+ softmax issue) and part2 (transpose + AV + fp8 re-quantize) issued ~2
projection units later so the vector-chain latency hides under matmuls.
"""

import json
import os
from functools import lru_cache

import numpy as np
import ml_dtypes

import concourse.bass as bass
import concourse.mybir as mybir
import concourse.tile as tile
from concourse.bass_utils import run_bass_kernel_spmd


def _split_waits_json(bir: bytes) -> bytes:
    """Walrus in this container embeds at most 1 sem-wait per instruction
    (2 for EventSemaphore). Tile freely attaches more. Spill the excess
    onto same-engine NoOps inserted right before the instruction."""
    j = json.loads(bir)
    ctr = [0]

    def cap_of(op):
        return 2 if op == "EventSemaphore" else 1

    for f in j["functions"]:
        for blk in f["blocks"]:
            out = []
            for inst in blk["instructions"]:
                si = inst.get("sync_info")
                waits = (si or {}).get("on_wait") or []
                cap = cap_of(inst.get("opcode"))
                if len(waits) > cap:
                    extra, keep = waits[:-cap], waits[-cap:]
                    for w in extra:
                        ctr[0] += 1
                        out.append({
                            "debug": inst.get("debug", 0),
                            "engine": inst["engine"],
                            "ins": [], "outs": [],
                            "name": f"I-wspill-{ctr[0]}",
                            "opcode": "NoOp",
                            "sync_info": {"on_update": [], "on_wait": [w]},
                        })
                    si["on_wait"] = keep
                out.append(inst)
            blk["instructions"] = out
    return json.dumps(j).encode()


def _patch_to_json(nc):
    orig = nc.to_json_bytes
    nc.to_json_bytes = lambda: _split_waits_json(orig())
    return nc

F32 = mybir.dt.float32
F8 = mybir.dt.float8e4
BF16 = mybir.dt.bfloat16
AX = mybir.AxisListType
ALU = mybir.AluOpType
ACTF = mybir.ActivationFunctionType
DR = mybir.MatmulPerfMode.DoubleRow

B, S, D = 4, 4096, 2048
H, HD, W = 16, 128, 16
E = H * HD  # 2048
NCORES = 8
TOK_PER_CORE = B * S // NCORES  # 2048
TBLK = 512            # tokens per block
NBLK = TOK_PER_CORE // TBLK  # 4
KT = D // 128         # 16 contraction tiles
ET = E // 128         # 16 e-tiles (= heads)
NG = TBLK // 128      # 4 groups (of 8 windows) per block
MASK_NEG = -30000.0
A_SCALE = 32.0        # weight pre-scale so w_lo stays in e4m3 normal range


def build_kernel(nblk=NBLK):
    nc = bass.Bass("TRN2", target_bir_lowering=False, debug=False)

    ntok = nblk * TBLK
    # DRAM I/O (per core). fp8 operands come as interleaved hi/lo pairs:
    # x/o slots = (lo, hi); weight slots = (hi, lo) -> a "cross" DoubleRow
    # instruction pairing both slot-dims yields w_hi@x_lo + w_lo@x_hi.
    xcd = nc.dram_tensor("xc", [nblk, 128, KT, 2, TBLK], F8, kind="ExternalInput")
    wqcd = nc.dram_tensor("wqc", [ET, 128, KT, 2, 128], F8, kind="ExternalInput")
    wkcd = nc.dram_tensor("wkc", [ET, 128, KT, 2, 128], F8, kind="ExternalInput")
    wvcd = nc.dram_tensor("wvc", [E // 512, 128, KT, 2, 512], F8, kind="ExternalInput")
    wocd = nc.dram_tensor("woc", [D // 512, 128, ET, 2, 512], F8, kind="ExternalInput")
    csq = nc.dram_tensor("csq", [128, TBLK], BF16, kind="ExternalInput")
    snq = nc.dram_tensor("snq", [128, TBLK], BF16, kind="ExternalInput")
    csk = nc.dram_tensor("csk", [128, TBLK], BF16, kind="ExternalInput")
    snk = nc.dram_tensor("snk", [128, TBLK], BF16, kind="ExternalInput")
    maskd = nc.dram_tensor("maskd", [128, 4, 128], F32, kind="ExternalInput")
    idend = nc.dram_tensor("idend", [128, 128], BF16, kind="ExternalInput")
    outd = nc.dram_tensor("out", [ntok, D], BF16, kind="ExternalOutput")

    with tile.TileContext(nc) as tc:
        with (
            tc.tile_pool(name="const", bufs=1) as constp,
            tc.tile_pool(name="x", bufs=1) as xpool,
            tc.tile_pool(name="wqk", bufs=4) as wqkp,
            tc.tile_pool(name="wvo", bufs=3) as wvop,
            tc.tile_pool(name="qk", bufs=2) as qkpool,
            tc.tile_pool(name="v", bufs=1) as vpool,
            tc.tile_pool(name="oc", bufs=1) as ocpool,
            tc.tile_pool(name="rope", bufs=2) as ropep,
            tc.tile_pool(name="attn", bufs=2) as attnp,
            tc.tile_pool(name="small", bufs=4) as smallp,
            tc.tile_pool(name="osb", bufs=2) as osbp,
            tc.tile_pool(name="psA", bufs=3, space="PSUM") as psA,
            tc.tile_pool(name="psS", bufs=2, space="PSUM") as psS,
            tc.tile_pool(name="psT", bufs=1, space="PSUM") as psT,
            tc.tile_pool(name="psO", bufs=2, space="PSUM") as psO,
        ):
            # constants
            cs_q = constp.tile([128, TBLK], BF16, tag="csq")
            sn_q = constp.tile([128, TBLK], BF16, tag="snq")
            cs_k = constp.tile([128, TBLK], BF16, tag="csk")
            sn_k = constp.tile([128, TBLK], BF16, tag="snk")
            mask = constp.tile([128, 4, 128], F32, tag="mask")
            iden = constp.tile([128, 128], BF16, tag="iden")

            def make_qk(xt, qrt, krt):
                """Return (preload, units): the first PRE weight loads can
                be issued early (previous iteration) via preload(); each
                unit issues the load for unit i+PRE, 24 DoubleRow matmuls,
                and the RoPE copy-out. 32 units per block."""
                PRE = 3
                seq = []
                for wdram, cs_t, sn_t, dest in (
                    (wqcd, cs_q, sn_q, qrt),
                    (wkcd, cs_k, sn_k, krt),
                ):
                    for et in range(ET):
                        seq.append((wdram, cs_t, sn_t, dest, et))
                tiles = {}

                def load(i):
                    wt = wqkp.tile([128, KT, 2, 128], F8, tag="wqk")
                    nc.scalar.dma_start(wt[:], seq[i][0][seq[i][4]])
                    tiles[i] = wt

                def preload():
                    for i in range(PRE):
                        load(i)

                def units():
                    for i, (wdram, cs_t, sn_t, dest, et) in enumerate(seq):
                        def unit(i=i, cs_t=cs_t, sn_t=sn_t, dest=dest, et=et):
                            if i + PRE < len(seq):
                                load(i + PRE)
                            wt = tiles.pop(i)
                            ps = psA.tile([128, TBLK], F32, tag="proj")
                            for j in range(KT // 2):
                                nc.tensor.matmul(
                                    ps[:], wt[:, 2 * j:2 * j + 2, 0, :],
                                    xt[:, 2 * j:2 * j + 2, 1, :],
                                    start=(j == 0), stop=False, perf_mode=DR)
                            for k in range(KT):
                                nc.tensor.matmul(
                                    ps[:], wt[:, k, :, :], xt[:, k, :, :],
                                    start=False, stop=(k == KT - 1),
                                    perf_mode=DR)
                            # RoPE in bf16: qb = bf16(ps) on ACT, then
                            # dest = qb*cs + swap64(qb)*sn on DVE (2x mode)
                            qb = ropep.tile([128, TBLK], BF16, tag="qb")
                            nc.scalar.activation(qb[:], ps[:], ACTF.Copy)
                            # sn tables are half-swapped on host so in0/in1
                            # share a base partition (walrus SB constraint)
                            rot = ropep.tile([128, TBLK], BF16, tag="rot")
                            qcs = ropep.tile([128, TBLK], BF16, tag="qcs")
                            nc.vector.tensor_tensor(
                                out=rot[0:64, :], in0=qb[64:128, :],
                                in1=sn_t[64:128, :], op=ALU.mult)
                            nc.vector.tensor_tensor(
                                out=rot[64:128, :], in0=qb[0:64, :],
                                in1=sn_t[0:64, :], op=ALU.mult)
                            nc.vector.tensor_tensor(
                                out=qcs[:], in0=qb[:], in1=cs_t[:],
                                op=ALU.mult)
                            nc.vector.tensor_tensor(
                                out=dest[:, et, :], in0=qcs[:], in1=rot[:],
                                op=ALU.add)
                        yield unit

                return preload, units

            def v_units(xt, vt):
                """Yield one closure per (ec, tt) V-projection psum tile.
                Weight chunk ec+1 is prefetched at (ec, tt=0); loads are
                split in halves so matmuls start on the first piece."""
                tiles = {}

                def load(ec):
                    wv = wvop.tile([128, KT, 2, 512], F8, tag="wvo")
                    for q in range(4):
                        nc.scalar.dma_start(
                            wv[:, q * 4:(q + 1) * 4, :, :],
                            wvcd[ec][:, q * 4:(q + 1) * 4, :, :])
                    tiles[ec] = wv

                for ec in range(E // 512):
                    for tt in range(NG):
                        def unit(ec=ec, tt=tt):
                            if ec == 0 and tt == 0:
                                load(0)
                                load(1)
                            if tt == 0 and ec + 2 < E // 512:
                                load(ec + 2)
                            wv = tiles[ec]
                            ps = psA.tile([128, 512], F32, tag="proj")
                            for j in range(KT // 2):
                                nc.tensor.matmul(
                                    ps[:],
                                    xt[:, 2 * j:2 * j + 2, 1,
                                       tt * 128:(tt + 1) * 128],
                                    wv[:, 2 * j:2 * j + 2, 0, :],
                                    start=(j == 0), stop=False, perf_mode=DR)
                            for k in range(KT):
                                nc.tensor.matmul(
                                    ps[:],
                                    xt[:, k, :, tt * 128:(tt + 1) * 128],
                                    wv[:, k, :, :],
                                    start=False, stop=(k == KT - 1),
                                    perf_mode=DR)
                            nc.scalar.activation(
                                vt[:, tt, ec * 512:(ec + 1) * 512], ps[:],
                                ACTF.Copy)
                        yield unit

            def attn_quads(qrt, krt, vt, oct_):
                """Yield (part1, part2) closures per (g, h0) quad."""
                for g in range(NG):
                    gs = g * 128
                    for h0 in range(0, H, 4):
                        state = {}

                        def part1(g=g, gs=gs, h0=h0, state=state):
                            sps = psS.tile([128, 4, 128], F32, tag="s")
                            for i in range(4):
                                h = h0 + i
                                nc.tensor.matmul(
                                    sps[:, i, :], qrt[:, h, gs:gs + 128],
                                    krt[:, h, gs:gs + 128],
                                    start=True, stop=True)
                            sm = attnp.tile([128, 4, 128], F32, tag="sm")
                            nc.vector.tensor_tensor(
                                out=sm[:], in0=sps[:], in1=mask[:],
                                op=ALU.add)
                            pt = attnp.tile([128, 4, 128], BF16, tag="pt")
                            sums = smallp.tile([128, 4], F32, tag="sums")
                            for i in range(4):
                                nc.scalar.activation(
                                    pt[:, i, :], sm[:, i, :], ACTF.Exp,
                                    accum_out=sums[:, i:i + 1])
                            rec = smallp.tile([128, 4], F32, tag="rec")
                            nc.vector.reciprocal(rec[:], sums[:])
                            at = attnp.tile([128, 4, 128], BF16, tag="at")
                            for i in range(4):
                                nc.vector.tensor_scalar_mul(
                                    at[:, i, :], pt[:, i, :], rec[:, i:i + 1])
                            state["at"] = at

                        def part2(g=g, gs=gs, h0=h0, state=state):
                            at = state["at"]
                            atps = psT.tile([128, 4, 128], BF16, tag="t")
                            for i in range(4):
                                nc.tensor.transpose(
                                    atps[:, i, :], at[:, i, :], iden[:])
                            ats = attnp.tile([128, 4, 128], BF16, tag="ats")
                            nc.vector.tensor_copy(ats[:], atps[:])
                            ops_ = psO.tile([128, 4, 128], F32, tag="o")
                            for i in range(4):
                                h = h0 + i
                                nc.tensor.matmul(
                                    ops_[:, i, :],
                                    vt[:, g, h * 128:(h + 1) * 128],
                                    ats[:, i, :], start=True, stop=True)
                            # quantize A*outT to fp8 pair: hi then lo residual
                            nc.scalar.activation(
                                oct_[:, h0:h0 + 4, 1, gs:gs + 128], ops_[:],
                                ACTF.Copy)
                            nc.vector.tensor_tensor(
                                out=oct_[:, h0:h0 + 4, 0, gs:gs + 128],
                                in0=ops_[:],
                                in1=oct_[:, h0:h0 + 4, 1, gs:gs + 128],
                                op=ALU.subtract)

                        yield part1, part2

            def o_units(oct_, ts):
                """Yield one closure per (dc, tt) O-projection psum tile."""
                tiles = {}

                def load(dc):
                    wo = wvop.tile([128, ET, 2, 512], F8, tag="wvo")
                    for q in range(4):
                        nc.scalar.dma_start(
                            wo[:, q * 4:(q + 1) * 4, :, :],
                            wocd[dc][:, q * 4:(q + 1) * 4, :, :])
                    tiles[dc] = wo

                for dc in range(D // 512):
                    for tt in range(NG):
                        def unit(dc=dc, tt=tt):
                            if dc == 0 and tt == 0:
                                load(0)
                                load(1)
                            if tt == 0 and dc + 2 < D // 512:
                                load(dc + 2)
                            wo = tiles[dc]
                            ps = psA.tile([128, 512], F32, tag="proj")
                            for j in range(ET // 2):
                                nc.tensor.matmul(
                                    ps[:],
                                    oct_[:, 2 * j:2 * j + 2, 1,
                                         tt * 128:(tt + 1) * 128],
                                    wo[:, 2 * j:2 * j + 2, 0, :],
                                    start=(j == 0), stop=False, perf_mode=DR)
                            for et in range(ET):
                                nc.tensor.matmul(
                                    ps[:],
                                    oct_[:, et, :, tt * 128:(tt + 1) * 128],
                                    wo[:, et, :, :],
                                    start=False, stop=(et == ET - 1),
                                    perf_mode=DR)
                            osb = osbp.tile([128, 512], BF16, tag="osb")
                            nc.scalar.activation(
                                osb[:], ps[:], ACTF.Copy,
                                scale=1.0 / (A_SCALE * A_SCALE))
                            nc.sync.dma_start(
                                outd[ts + tt * 128: ts + (tt + 1) * 128,
                                     dc * 512:(dc + 1) * 512],
                                osb[:],
                            )
                        yield unit

            def load_x(b):
                xt = xpool.tile([128, KT, 2, TBLK], F8, tag="xc")
                for q in range(4):
                    nc.sync.dma_start(
                        xt[:, q * 4:(q + 1) * 4, :, :],
                        xcd[b][:, q * 4:(q + 1) * 4, :, :])
                return xt

            # x(0) ahead of the constants so the first matmuls start early
            xt_next = load_x(0)
            nc.sync.dma_start(cs_q[:], csq[:])
            nc.sync.dma_start(sn_q[:], snq[:])
            nc.sync.dma_start(cs_k[:], csk[:])
            nc.sync.dma_start(sn_k[:], snk[:])
            nc.sync.dma_start(mask[:], maskd[:])
            nc.sync.dma_start(iden[:], idend[:])

            def prep_block(bb):
                """Create block bb's q/k tiles + weight preloader."""
                qrt = qkpool.tile([128, ET, TBLK], BF16, tag="qrt")
                krt = qkpool.tile([128, ET, TBLK], BF16, tag="krt")
                pre, us = make_qk(xt_next, qrt, krt)
                pre()
                return qrt, krt, us

            # ---- software-pipelined block loop: attention of block b-1
            # rides under the projection units of block b; the last block's
            # attention rides under its own O-projection.
            nxt = prep_block(0)
            prev = None  # ((qrt, krt, vt, oct_), oct_, ts_prev)
            for b in range(nblk + 1):
                units = []
                if b < nblk:
                    xt = xt_next
                    qrt, krt, qk_us = nxt
                    vt = vpool.tile([128, NG, E], BF16, tag="vt")
                    units.extend(qk_us())
                    units.extend(v_units(xt, vt))

                if prev is not None:
                    quads = list(attn_quads(*prev[0]))
                    o_us = list(o_units(prev[1], prev[2]))
                    if units:
                        pend = []
                        qi = 0
                        for ui, unit in enumerate(units):
                            unit()
                            if ui % 3 == 2:
                                if pend:
                                    pend.pop(0)[1]()
                                if qi < len(quads):
                                    quads[qi][0]()
                                    pend.append(quads[qi])
                                    qi += 1
                        while qi < len(quads) or pend:
                            if pend:
                                pend.pop(0)[1]()
                            if qi < len(quads):
                                quads[qi][0]()
                                pend.append(quads[qi])
                                qi += 1
                        if b + 1 < nblk:
                            xt_next = load_x(b + 1)
                            nxt = prep_block(b + 1)
                        for u in o_us:
                            u()
                    else:
                        # final iteration: interleave the quads with the
                        # first wo chunk's O units (unit dc0/tt=g becomes
                        # ready once group g's quads complete).
                        pend = []
                        done = 0
                        for qi, quad in enumerate(quads):
                            quad[0]()
                            if pend:
                                pend.pop(0)[1]()
                                done += 1
                                if done % 4 == 0:
                                    o_us[done // 4 - 1]()
                            pend.append(quad)
                        while pend:
                            pend.pop(0)[1]()
                            done += 1
                            if done % 4 == 0:
                                o_us[done // 4 - 1]()
                        for u in o_us[NG:]:
                            u()
                else:
                    for unit in units:
                        unit()
                    if nblk > 1:
                        xt_next = load_x(1)
                        nxt = prep_block(1)

                if b < nblk:
                    oct_ = ocpool.tile([128, ET, 2, TBLK], F8, tag="oc")
                    prev = ((qrt, krt, vt, oct_), oct_, b * TBLK)
    return _patch_to_json(nc)


_F8NP = ml_dtypes.float8_e4m3


def _split_f8(a):
    hi = a.astype(_F8NP)
    lo = (a - hi.astype(np.float32)).astype(_F8NP)
    return hi, lo


def _interleave(arr, lohi):
    """arr [KTlike, 128, C0, C1] fp32 -> [C0, 128, KTlike, 2, C1] fp8 with
    slot order (hi, lo) when lohi=False else (lo, hi)."""
    hi, lo = _split_f8(arr)
    pair = (lo, hi) if lohi else (hi, lo)
    st = np.stack(pair, axis=-1)  # [KT, 128, C0, C1, 2]
    return np.ascontiguousarray(st.transpose(2, 1, 0, 4, 3))


def _host_prep(x, rope_freqs, wq, wk, wv, wo):
    A = A_SCALE
    x_flat = np.ascontiguousarray(x, dtype=np.float32).reshape(B * S, D)

    def w_c(w, c1):
        wT = np.ascontiguousarray(w.T, dtype=np.float32) * A
        kt = wT.shape[0] // 128
        arr = wT.reshape(kt, 128, wT.shape[1] // c1, c1)
        return _interleave(arr, lohi=False)

    wqc = w_c(wq, 128)
    wkc = w_c(wk, 128)
    wvc = w_c(wv, 512)
    woc = w_c(wo, 512)

    f = np.asarray(rope_freqs[:W], dtype=np.float32)  # [16, 64]
    cosf, sinf = np.cos(f), np.sin(f)                 # [16, 64]
    tmod = np.arange(TBLK) % W
    cs = np.empty((128, TBLK), np.float32)
    sn = np.empty((128, TBLK), np.float32)
    p = np.arange(128)
    cs[:, :] = cosf[tmod[None, :], (p % 64)[:, None]]
    sn[:, :] = sinf[tmod[None, :], (p % 64)[:, None]]
    sn[0:64, :] *= -1.0
    scale = 1.0 / np.sqrt(np.float32(HD))
    bf = ml_dtypes.bfloat16
    # halves pre-swapped: kernel computes rot[0:64] = qb[64:128]*sn_sw[64:128]
    sn_sw = np.concatenate([sn[64:128], sn[0:64]], axis=0)
    csq, snq = (cs * (scale / A)).astype(bf), (sn_sw * (scale / A)).astype(bf)
    csk, snk = (cs * (1.0 / A)).astype(bf), (sn_sw * (1.0 / A)).astype(bf)

    maskm = np.full((128, 128), MASK_NEG, np.float32)
    for wdw in range(128 // W):
        maskm[wdw * W:(wdw + 1) * W, wdw * W:(wdw + 1) * W] = 0.0
    maskm = np.repeat(maskm[:, None, :], 4, axis=1).copy()
    iden = np.eye(128, dtype=ml_dtypes.bfloat16)

    shared = dict(wqc=wqc, wkc=wkc, wvc=wvc, woc=woc,
                  csq=csq, snq=snq, csk=csk, snk=snk,
                  maskd=maskm, idend=iden)
    in_maps = []
    for c in range(NCORES):
        shard = x_flat[c * TOK_PER_CORE:(c + 1) * TOK_PER_CORE]
        xT = np.ascontiguousarray(shard.T)  # [D, 2048]
        arr = xT.reshape(KT, 128, NBLK, TBLK)
        xc = _interleave(arr, lohi=True)  # [NBLK, 128, KT, 2, TBLK]
        in_maps.append(dict(shared, xc=xc))
    return in_maps


@lru_cache(maxsize=1)
def _get_nc():
    return build_kernel()


def kernel(x, rope_freqs, wq, wk, wv, wo):
    in_maps = _host_prep(x, rope_freqs, wq, wk, wv, wo)
    nc = _get_nc()
    res = run_bass_kernel_spmd(
        nc, in_maps, core_ids=list(range(NCORES)),
        trace=bool(int(os.environ.get("LWA_TRACE", "0"))),
    )
    if getattr(kernel, "_last_results", None) is not None or True:
        kernel._last_results = res
    out = np.concatenate(
        [np.asarray(r["out"], dtype=np.float32) for r in res.results], axis=0)
    return out.reshape(B, S, D)
